# revision 1
# baseline (speedup 1.0000x reference)
"""Trainium2 Bass kernel for nn_LoRAConvsByRandom.

Strategy (hardcoded for the [16, 704, 68, 68] problem):
  - Shard the 64 channel-groups across 8 cores (8 groups/core), all 16 samples.
  - The whole computation (4-rep permutation gather-sum + 11-branch shift-add
    + crop) is linear in x, so per (group, direction) it is ONE matmul:
        out1[t, (b,w)] = sum_{(j,h)} W1[(j,h), t] * x[g, j, h, (b, w+2)]
    with W1 built on the host from idx1 (counts of (branch i, channel j) pairs,
    nonzero where h = t - 21 + 5i).  small_x rides in spare lhsT columns
    (m = 64..127) of the same matmul.  Direction 2 mixes along w instead of h,
    so it uses a host-pretransposed copy of x (rows = (c, w), free = (b, h))
    and produces out2 transposed ([w, (b, t)]); the host untransposes.
  - Data in bf16 (weights are small exact integers), PSUM accumulates f32,
    outputs stored bf16 and upcast on host.
  - DMA strategy (the kernel is HBM-bound: ~30.6MB/core at ~360GB/s/core):
    host packs ONE contiguous slab per group [128p, 210*64] holding
    [w1 | w2 | x0 half0 | x1 half0 | x0 half1 | x1 half1], so the whole
    input streams as 8 fat DMAs (26.9KB per partition row) that are all
    issued up front (x pool bufs=7) — few instructions keeps the 16 DMA
    queues saturated.  The last group ships as three pieces (w+half0 /
    x0half1+x1half1-kt0:3 / x1half1-kt4:5, separate ring tiles) so after
    the final byte lands only two matmuls + one copy + one small DMA
    remain; its outputs drain per 8-sample half with the final o1/o2
    dispatches split across the scalar and sync sequencers.  Output DMAs
    otherwise issue from the scalar sequencer so they never block input
    dispatch on the sync sequencer.
"""

import os
import numpy as np
import ml_dtypes

NK = 11
EXTRA = 2
B = 16
C_OUT = 64
C_IN = 704
HIN = 68
ORI = 64
N_CORES = 8
GPC = C_OUT // N_CORES           # 8 groups per core
ROWS_G = NK * HIN                # 748 rows per group
KT = 6                           # K-tiles of 128 rows (748 -> 768 zero-padded)
ROWS_CORE = GPC * ROWS_G         # 5984 real rows per core

STATS = {}
_CACHE = {}


def _dt():
    import concourse.mybir as mybir
    f32 = os.environ.get("KERNEL_F32", "0") == "1"
    return (mybir.dt.float32, np.float32) if f32 else (mybir.dt.bfloat16, ml_dtypes.bfloat16)


def _build_nc():
    import concourse.bass as bass
    import concourse.tile as tile
    from concourse import bacc
    import concourse.mybir as mybir

    mdt, _ = _dt()

    nc = bacc.Bacc(None, target_bir_lowering=False, debug=False)
    # One slab per group, unit-64 columns:
    #   [w1 12u | w2 6u | x0h0 48u | x1h0 48u | x0h1 48u | x1h1 48u]
    # (x dir 0 = (c,h)-rows w-cropped, dir 1 = (c,w)-rows h-cropped; h = b-half)
    U = 12 + 6 + 96 + 96
    xa = nc.declare_dram_parameter("xa", [GPC, 128, U, 64], mdt, isOutput=False)
    o1 = nc.declare_dram_parameter("o1", [GPC, 128, B, ORI], mdt, isOutput=True)
    o2 = nc.declare_dram_parameter("o2", [GPC, 64, B, ORI], mdt, isOutput=True)

    with tile.TileContext(nc) as tc:
        with (
            tc.tile_pool(name="x", bufs=7) as xpool,
            tc.tile_pool(name="o", bufs=3) as opool,
            tc.tile_pool(name="p1", bufs=4, space=bass.MemorySpace.PSUM) as p1pool,
            tc.tile_pool(name="p2", bufs=4, space=bass.MemorySpace.PSUM) as p2pool,
        ):
            # column layout (units of 64): [w1 12 | w2 6 | x0h0 48 | x1h0 48 | x0h1 48 | x1h1 48]
            # last group ships as three pieces (w+h0, x0h1, x1h1) so its
            # compute overlaps its own DMA and shortens the tail
            slabs = []
            last = GPC - 1
            for gl in range(last):
                s = xpool.tile([128, U, 64], mdt, tag="s")
                nc.sync.dma_start(out=s[:], in_=xa[gl])
                slabs.append(s)
            sA = xpool.tile([128, U, 64], mdt, tag="s")
            nc.sync.dma_start(out=sA[:, 0:114, :], in_=xa[last, :, 0:114])
            sB = xpool.tile([128, U, 64], mdt, tag="s")
            nc.sync.dma_start(out=sB[:, 0:88, :], in_=xa[last, :, 114:202])
            sC = xpool.tile([128, U, 64], mdt, tag="s")
            nc.sync.dma_start(out=sC[:, 0:8, :], in_=xa[last, :, 202:210])
            slabs.append(sA)

            for gl in range(GPC):
                s = slabs[gl]
                o1g = opool.tile([128, B, ORI], mdt, tag="o1")
                o2g = opool.tile([64, B, ORI], mdt, tag="o2")
                for half in range(2):
                    b0 = half * 8
                    if gl == last and half == 1:
                        def rh(dir_, kt):
                            c = dir_ * 48 + kt * 8
                            if c < 88:
                                return sB[:, c:c + 8, :]
                            return sC[:, c - 88:c - 80, :]
                    else:
                        c0 = 18 + half * 96
                        rh = lambda dir_, kt: s[:, c0 + dir_ * 48 + kt * 8:c0 + dir_ * 48 + kt * 8 + 8, :]
                    ps1 = p1pool.tile([128, 8, ORI], mybir.dt.float32, tag="ps1")
                    for kt in range(KT):
                        nc.tensor.matmul(
                            ps1[:],
                            s[:, kt * 2:kt * 2 + 2, :],
                            rh(0, kt),
                            start=(kt == 0),
                            stop=(kt == KT - 1),
                        )
                    nc.vector.tensor_copy(o1g[:, b0:b0 + 8, :], ps1[:])

                    ps2 = p2pool.tile([64, 8, ORI], mybir.dt.float32, tag="ps2")
                    for kt in range(KT):
                        nc.tensor.matmul(
                            ps2[:],
                            s[:, 12 + kt, :],
                            rh(1, kt),
                            start=(kt == 0),
                            stop=(kt == KT - 1),
                        )
                    nc.scalar.copy(o2g[:, b0:b0 + 8, :], ps2[:])
                    if gl == last:
                        nc.scalar.dma_start(out=o1[gl, :, b0:b0 + 8, :], in_=o1g[:, b0:b0 + 8, :])
                        nc.sync.dma_start(out=o2[gl, :, b0:b0 + 8, :], in_=o2g[:, b0:b0 + 8, :])
                if gl < last:
                    nc.scalar.dma_start(out=o1[gl], in_=o1g[:])
                    nc.scalar.dma_start(out=o2[gl], in_=o2g[:])
    nc.compile()
    return nc


def _get_nc():
    key = os.environ.get("KERNEL_F32", "0")
    if key not in _CACHE:
        _CACHE[key] = _build_nc()
    return _CACHE[key]


def _counts(idx):
    """idx [n_rep, 704] -> c[g, i, j] = #(r: idx[r, g*11+i] == g*11+j)."""
    c = np.zeros((C_OUT, NK, NK), np.float32)
    for r in range(idx.shape[0]):
        p = idx[r].reshape(C_OUT, NK) - np.arange(C_OUT)[:, None] * NK
        for g in range(C_OUT):
            for i in range(NK):
                c[g, i, p[g, i]] += 1
    return c


def _build_weights(idx1, idx2, idx_small):
    c1 = _counts(idx1)
    c2 = _counts(idx2)
    scnt = np.zeros((C_OUT, NK), np.float32)
    for r in range(idx_small.shape[0]):
        j = idx_small[r] - np.arange(C_OUT) * NK
        for g in range(C_OUT):
            scnt[g, j[g]] += 1

    w1 = np.zeros((C_OUT, KT * 128, 128), np.float32)
    w2 = np.zeros((C_OUT, KT * 128, 64), np.float32)
    for t in range(ORI):
        for i in range(NK):
            h = t - 21 + 5 * i
            if 0 <= h < HIN:
                w1[:, np.arange(NK) * HIN + h, t] += c1[:, i, :]
                w2[:, np.arange(NK) * HIN + h, t] += c2[:, i, :]
    for tp in range(ORI):
        w1[:, np.arange(NK) * HIN + (tp + EXTRA), 64 + tp] = scnt
    return w1, w2


def _ensure_ntff_hook():
    """Register the axon NTFF profile hook if the container's antenv lacks it."""
    import sys
    import types
    try:
        from antenv.axon_hooks import get_axon_ntff_profile_hook  # noqa: F401
        return
    except ImportError:
        pass
    try:
        import antenv
        from trn_agent_boot.trn_boot import _ntff_profile_via_ctypes
        mod = types.ModuleType("antenv.axon_hooks")
        _h = [None]
        mod.set_axon_ntff_profile_hook = lambda hook: _h.__setitem__(0, hook)
        mod.get_axon_ntff_profile_hook = lambda: _h[0]
        sys.modules["antenv.axon_hooks"] = mod
        antenv.axon_hooks = mod
        hook = _ntff_profile_via_ctypes("/opt/axon/libaxon_pjrt.so")
        if hook is not None:
            mod.set_axon_ntff_profile_hook(hook)
    except Exception:
        pass


def kernel(inputs, idx1, idx2, idx_small, ori_h=64, ori_w=64):
    from concourse.bass_utils import run_bass_kernel_spmd

    x = np.asarray(inputs, dtype=np.float32)
    idx1 = np.asarray(idx1)
    idx2 = np.asarray(idx2)
    idx_small = np.asarray(idx_small)
    _, npdt = _dt()

    xb = x.astype(npdt)
    # rows (c,h), free (b, w in [2,66))  /  rows (c,w), free (b, h in [2,66))
    xr_all = np.ascontiguousarray(
        xb.transpose(1, 2, 0, 3)[:, :, :, EXTRA:EXTRA + ORI]
    ).reshape(C_IN * HIN, B, ORI)
    xtr_all = np.ascontiguousarray(
        xb.transpose(1, 3, 0, 2)[:, :, :, EXTRA:EXTRA + ORI]
    ).reshape(C_IN * HIN, B, ORI)
    w1_all, w2_all = _build_weights(idx1, idx2, idx_small)

    in_maps = []
    for c in range(N_CORES):
        # per-group slabs: [gl, p, 210, 64] with the unit-64 column layout above
        pad = np.zeros((GPC * ROWS_G + 20, B, ORI), npdt)
        padt = np.zeros_like(pad)
        pad[:ROWS_CORE] = xr_all[c * ROWS_CORE:(c + 1) * ROWS_CORE]
        padt[:ROWS_CORE] = xtr_all[c * ROWS_CORE:(c + 1) * ROWS_CORE]
        w1c = w1_all[c * GPC:(c + 1) * GPC].reshape(GPC, KT, 128, 128).transpose(0, 2, 1, 3)
        w2c = w2_all[c * GPC:(c + 1) * GPC].reshape(GPC, KT, 128, 64).transpose(0, 2, 1, 3)
        xa = np.empty((GPC, 128, 210, 64), npdt)
        for gl in range(GPC):
            sl = slice(gl * ROWS_G, gl * ROWS_G + KT * 128)
            xa[gl, :, 0:12] = w1c[gl].reshape(128, 12, 64).astype(npdt)
            xa[gl, :, 12:18] = w2c[gl].reshape(128, 6, 64).astype(npdt)
            x0g = pad[sl].reshape(KT, 128, 2, 8, ORI).transpose(1, 2, 0, 3, 4)
            x1g = padt[sl].reshape(KT, 128, 2, 8, ORI).transpose(1, 2, 0, 3, 4)
            xa[gl, :, 18:66] = x0g[:, 0].reshape(128, 48, 64)
            xa[gl, :, 66:114] = x1g[:, 0].reshape(128, 48, 64)
            xa[gl, :, 114:162] = x0g[:, 1].reshape(128, 48, 64)
            xa[gl, :, 162:210] = x1g[:, 1].reshape(128, 48, 64)
        in_maps.append({"xa": xa})

    nc = _get_nc()
    trace = os.environ.get("KERNEL_TRACE", "0") == "1"
    if trace:
        _ensure_ntff_hook()
        try:
            br = run_bass_kernel_spmd(nc, in_maps, core_ids=list(range(N_CORES)), trace=True)
        except Exception as e:
            print(f"[kernel] traced run failed ({type(e).__name__}: {e}); retrying untraced")
            br = run_bass_kernel_spmd(nc, in_maps, core_ids=list(range(N_CORES)), trace=False)
    else:
        br = run_bass_kernel_spmd(nc, in_maps, core_ids=list(range(N_CORES)), trace=False)
    STATS["exec_time_ns"] = br.exec_time_ns
    STATS["mean_exec_time_ns"] = br.mean_exec_time_ns
    STATS["profile_json"] = br.profile_json

    o1 = np.stack([br.results[c]["o1"] for c in range(N_CORES)]).reshape(C_OUT, 128, B, ORI)
    o2 = np.stack([br.results[c]["o2"] for c in range(N_CORES)]).reshape(C_OUT, 64, B, ORI)
    out1 = np.ascontiguousarray(o1[:, :64].transpose(2, 0, 1, 3).astype(np.float32))
    small = np.ascontiguousarray(o1[:, 64:].transpose(2, 0, 1, 3).astype(np.float32))
    out2 = np.ascontiguousarray(o2.transpose(2, 0, 3, 1).astype(np.float32))
    return out1, out2, small



# revision 8
# speedup vs baseline: 1.3710x; 1.3710x over previous
"""Trainium2 Bass kernel for nn_LoRAConvsByRandom.

Strategy (hardcoded for the [16, 704, 68, 68] problem):
  - Shard the 64 channel-groups across 8 cores (8 groups/core), all 16 samples.
  - The whole computation (4-rep permutation gather-sum + 11-branch shift-add
    + crop) is linear in x, so per (group, direction) it is ONE matmul:
        out1[t, (b,w)] = sum_{(j,h)} W1[(j,h), t] * x[g, j, h, (b, w+2)]
    with W1 built on the host from idx1 (counts of (branch i, channel j) pairs,
    nonzero where h = t - 21 + 5i).  small_x rides in spare lhsT columns
    (m = 64..127) of the same matmul.  Direction 2 mixes along w instead of h,
    so it uses a host-pretransposed copy of x (rows = (c, w), free = (b, h))
    and produces out2 transposed ([w, (b, t)]); the host untransposes.
  - Data in fp8 e3m4 (x pre-scaled by S, weights are small exact integers
    that e3m4 represents exactly), PSUM accumulates f32, dequant by 1/S on
    the PSUM->SBUF copy, outputs stored bf16 and upcast on host.  The
    idx_small-selected channels of the dir-1 copy are quantized with
    error-feedback so the 4-term `small` sum keeps ~1-quantum error.
  - DMA strategy (the kernel is HBM-bound: ~16.9MB/core at ~360GB/s/core):
    host packs ONE contiguous slab per group [128p, 210*64] holding
    [w1 | w2 | x0 half0 | x1 half0 | x0 half1 | x1 half1], so the whole
    input streams as 8 fat DMAs (26.9KB per partition row) that are all
    issued up front (x pool bufs=7) — few instructions keeps the 16 DMA
    queues saturated.  The last group ships as three pieces (w+half0 /
    x0half1+x1half1-kt0:3 / x1half1-kt4:5, separate ring tiles) so after
    the final byte lands only two matmuls + one copy + one small DMA
    remain; its outputs drain per 8-sample half with the final o1/o2
    dispatches split across the scalar and sync sequencers.  Output DMAs
    otherwise issue from the scalar sequencer so they never block input
    dispatch on the sync sequencer.
"""

import os
import numpy as np
import ml_dtypes

NK = 11
EXTRA = 2
B = 16
C_OUT = 64
C_IN = 704
HIN = 68
ORI = 64
N_CORES = 8
GPC = C_OUT // N_CORES           # 8 groups per core
ROWS_G = NK * HIN                # 748 rows per group
KT = 6                           # K-tiles of 128 rows (748 -> 768 zero-padded)
ROWS_CORE = GPC * ROWS_G         # 5984 real rows per core

STATS = {}
_CACHE = {}


S_QUANT = 1.45                   # pre-quantization scale for e3m4 binade placement


def _dt():
    """(x/weight dtype, numpy x dtype, output dtype, numpy output dtype)."""
    import concourse.mybir as mybir
    f32 = os.environ.get("KERNEL_F32", "0") == "1"
    if f32:
        return mybir.dt.float32, np.float32, mybir.dt.float32, np.float32
    return (mybir.dt.float8e3, ml_dtypes.float8_e3m4,
            mybir.dt.bfloat16, ml_dtypes.bfloat16)


def _build_nc():
    import concourse.bass as bass
    import concourse.tile as tile
    from concourse import bacc
    import concourse.mybir as mybir

    mdt, _, odt, _ = _dt()
    dq = 1.0 / S_QUANT if mdt != mybir.dt.float32 else 1.0

    nc = bacc.Bacc(None, target_bir_lowering=False, debug=False)
    # One slab per group, unit-64 columns:
    #   [w1 12u | w2 6u | x0h0 48u | x1h0 48u | x0h1 48u | x1h1 48u]
    # (x dir 0 = (c,h)-rows w-cropped, dir 1 = (c,w)-rows h-cropped; h = b-half)
    U = 12 + 6 + 96 + 96
    xa = nc.declare_dram_parameter("xa", [GPC, 128, U, 64], mdt, isOutput=False)
    o1 = nc.declare_dram_parameter("o1", [GPC, 128, B, ORI], odt, isOutput=True)
    o2 = nc.declare_dram_parameter("o2", [GPC, 64, B, ORI], odt, isOutput=True)

    with tile.TileContext(nc) as tc:
        with (
            tc.tile_pool(name="x", bufs=7) as xpool,
            tc.tile_pool(name="o", bufs=3) as opool,
            tc.tile_pool(name="p1", bufs=4, space=bass.MemorySpace.PSUM) as p1pool,
            tc.tile_pool(name="p2", bufs=4, space=bass.MemorySpace.PSUM) as p2pool,
        ):
            # column layout (units of 64): [w1 12 | w2 6 | x0h0 48 | x1h0 48 | x0h1 48 | x1h1 48]
            # last group ships as three pieces (w+h0, x0h1, x1h1) so its
            # compute overlaps its own DMA and shortens the tail
            slabs = []
            last = GPC - 1
            for gl in range(last):
                s = xpool.tile([128, U, 64], mdt, tag="s")
                nc.sync.dma_start(out=s[:], in_=xa[gl])
                slabs.append(s)
            sA = xpool.tile([128, U, 64], mdt, tag="s")
            nc.sync.dma_start(out=sA[:, 0:114, :], in_=xa[last, :, 0:114])
            sB = xpool.tile([128, U, 64], mdt, tag="s")
            nc.sync.dma_start(out=sB[:, 0:88, :], in_=xa[last, :, 114:202])
            sC = xpool.tile([128, U, 64], mdt, tag="s")
            nc.sync.dma_start(out=sC[:, 0:8, :], in_=xa[last, :, 202:210])
            slabs.append(sA)

            for gl in range(GPC):
                s = slabs[gl]
                o1g = opool.tile([128, B, ORI], odt, tag="o1")
                o2g = opool.tile([64, B, ORI], odt, tag="o2")
                for half in range(2):
                    b0 = half * 8
                    if gl == last and half == 1:
                        def rh(dir_, kt):
                            c = dir_ * 48 + kt * 8
                            if c < 88:
                                return sB[:, c:c + 8, :]
                            return sC[:, c - 88:c - 80, :]
                    else:
                        c0 = 18 + half * 96
                        rh = lambda dir_, kt: s[:, c0 + dir_ * 48 + kt * 8:c0 + dir_ * 48 + kt * 8 + 8, :]
                    ps1 = p1pool.tile([128, 8, ORI], mybir.dt.float32, tag="ps1")
                    for kt in range(KT):
                        nc.tensor.matmul(
                            ps1[:],
                            s[:, kt * 2:kt * 2 + 2, :],
                            rh(0, kt),
                            start=(kt == 0),
                            stop=(kt == KT - 1),
                        )
                    nc.vector.tensor_scalar_mul(o1g[:, b0:b0 + 8, :], ps1[:], dq)

                    ps2 = p2pool.tile([64, 8, ORI], mybir.dt.float32, tag="ps2")
                    for kt in range(KT):
                        nc.tensor.matmul(
                            ps2[:],
                            s[:, 12 + kt, :],
                            rh(1, kt),
                            start=(kt == 0),
                            stop=(kt == KT - 1),
                        )
                    nc.scalar.mul(o2g[:, b0:b0 + 8, :], ps2[:], dq)
                    if gl == last:
                        nc.scalar.dma_start(out=o1[gl, :, b0:b0 + 8, :], in_=o1g[:, b0:b0 + 8, :])
                        nc.sync.dma_start(out=o2[gl, :, b0:b0 + 8, :], in_=o2g[:, b0:b0 + 8, :])
                if gl < last:
                    nc.scalar.dma_start(out=o1[gl], in_=o1g[:])
                    nc.scalar.dma_start(out=o2[gl], in_=o2g[:])
    nc.compile()
    return nc


def _get_nc():
    key = os.environ.get("KERNEL_F32", "0")
    if key not in _CACHE:
        _CACHE[key] = _build_nc()
    return _CACHE[key]


def _counts(idx):
    """idx [n_rep, 704] -> c[g, i, j] = #(r: idx[r, g*11+i] == g*11+j)."""
    c = np.zeros((C_OUT, NK, NK), np.float32)
    for r in range(idx.shape[0]):
        p = idx[r].reshape(C_OUT, NK) - np.arange(C_OUT)[:, None] * NK
        for g in range(C_OUT):
            for i in range(NK):
                c[g, i, p[g, i]] += 1
    return c


def _build_weights(idx1, idx2, idx_small):
    c1 = _counts(idx1)
    c2 = _counts(idx2)
    scnt = np.zeros((C_OUT, NK), np.float32)
    for r in range(idx_small.shape[0]):
        j = idx_small[r] - np.arange(C_OUT) * NK
        for g in range(C_OUT):
            scnt[g, j[g]] += 1

    w1 = np.zeros((C_OUT, KT * 128, 128), np.float32)
    w2 = np.zeros((C_OUT, KT * 128, 64), np.float32)
    for t in range(ORI):
        for i in range(NK):
            h = t - 21 + 5 * i
            if 0 <= h < HIN:
                w1[:, np.arange(NK) * HIN + h, t] += c1[:, i, :]
                w2[:, np.arange(NK) * HIN + h, t] += c2[:, i, :]
    for tp in range(ORI):
        w1[:, np.arange(NK) * HIN + (tp + EXTRA), 64 + tp] = scnt
    return w1, w2


def _ensure_ntff_hook():
    """Register the axon NTFF profile hook if the container's antenv lacks it."""
    import sys
    import types
    try:
        from antenv.axon_hooks import get_axon_ntff_profile_hook  # noqa: F401
        return
    except ImportError:
        pass
    try:
        import antenv
        from trn_agent_boot.trn_boot import _ntff_profile_via_ctypes
        mod = types.ModuleType("antenv.axon_hooks")
        _h = [None]
        mod.set_axon_ntff_profile_hook = lambda hook: _h.__setitem__(0, hook)
        mod.get_axon_ntff_profile_hook = lambda: _h[0]
        sys.modules["antenv.axon_hooks"] = mod
        antenv.axon_hooks = mod
        hook = _ntff_profile_via_ctypes("/opt/axon/libaxon_pjrt.so")
        if hook is not None:
            mod.set_axon_ntff_profile_hook(hook)
    except Exception:
        pass


def kernel(inputs, idx1, idx2, idx_small, ori_h=64, ori_w=64):
    from concourse.bass_utils import run_bass_kernel_spmd

    x = np.asarray(inputs, dtype=np.float32)
    idx1 = np.asarray(idx1)
    idx2 = np.asarray(idx2)
    idx_small = np.asarray(idx_small)
    _, npdt, _, npodt = _dt()

    if npdt == np.float32:
        xq_a = x
        xq_b = x
    else:
        xs = x * S_QUANT
        xq_b = xs.astype(npdt)          # clean RNE: feeds dir-2 (lora2)
        xq_a = xq_b.copy()              # feeds dir-1 (lora1) + small
        # error-feedback quantization of the idx_small-selected channels so
        # the 4-term small sum keeps ~1-quantum error (channels re-quantized
        # in descending-multiplicity order, each absorbing the running
        # weighted residual of the previous ones)
        for g in range(C_OUT):
            js, counts = np.unique(idx_small[:, g], return_counts=True)
            order = np.argsort(-counts)
            r = np.zeros((B, HIN, HIN), np.float32)
            for k in order:
                c, m = int(js[k]), int(counts[k])
                qc = (xs[:, c] - r / m).astype(npdt)
                xq_a[:, c] = qc
                r += m * (qc.astype(np.float32) - xs[:, c])
    # rows (c,h), free (b, w in [2,66))  /  rows (c,w), free (b, h in [2,66))
    xr_all = np.ascontiguousarray(
        xq_a.transpose(1, 2, 0, 3)[:, :, :, EXTRA:EXTRA + ORI]
    ).reshape(C_IN * HIN, B, ORI)
    xtr_all = np.ascontiguousarray(
        xq_b.transpose(1, 3, 0, 2)[:, :, :, EXTRA:EXTRA + ORI]
    ).reshape(C_IN * HIN, B, ORI)
    w1_all, w2_all = _build_weights(idx1, idx2, idx_small)

    in_maps = []
    for c in range(N_CORES):
        # per-group slabs: [gl, p, 210, 64] with the unit-64 column layout above
        pad = np.zeros((GPC * ROWS_G + 20, B, ORI), npdt)
        padt = np.zeros_like(pad)
        pad[:ROWS_CORE] = xr_all[c * ROWS_CORE:(c + 1) * ROWS_CORE]
        padt[:ROWS_CORE] = xtr_all[c * ROWS_CORE:(c + 1) * ROWS_CORE]
        w1c = w1_all[c * GPC:(c + 1) * GPC].reshape(GPC, KT, 128, 128).transpose(0, 2, 1, 3)
        w2c = w2_all[c * GPC:(c + 1) * GPC].reshape(GPC, KT, 128, 64).transpose(0, 2, 1, 3)
        xa = np.empty((GPC, 128, 210, 64), npdt)
        for gl in range(GPC):
            sl = slice(gl * ROWS_G, gl * ROWS_G + KT * 128)
            xa[gl, :, 0:12] = w1c[gl].reshape(128, 12, 64).astype(npdt)
            xa[gl, :, 12:18] = w2c[gl].reshape(128, 6, 64).astype(npdt)
            x0g = pad[sl].reshape(KT, 128, 2, 8, ORI).transpose(1, 2, 0, 3, 4)
            x1g = padt[sl].reshape(KT, 128, 2, 8, ORI).transpose(1, 2, 0, 3, 4)
            xa[gl, :, 18:66] = x0g[:, 0].reshape(128, 48, 64)
            xa[gl, :, 66:114] = x1g[:, 0].reshape(128, 48, 64)
            xa[gl, :, 114:162] = x0g[:, 1].reshape(128, 48, 64)
            xa[gl, :, 162:210] = x1g[:, 1].reshape(128, 48, 64)
        in_maps.append({"xa": xa})

    nc = _get_nc()
    trace = os.environ.get("KERNEL_TRACE", "0") == "1"
    if trace:
        _ensure_ntff_hook()
        try:
            br = run_bass_kernel_spmd(nc, in_maps, core_ids=list(range(N_CORES)), trace=True)
        except Exception as e:
            print(f"[kernel] traced run failed ({type(e).__name__}: {e}); retrying untraced")
            br = run_bass_kernel_spmd(nc, in_maps, core_ids=list(range(N_CORES)), trace=False)
    else:
        br = run_bass_kernel_spmd(nc, in_maps, core_ids=list(range(N_CORES)), trace=False)
    STATS["exec_time_ns"] = br.exec_time_ns
    STATS["mean_exec_time_ns"] = br.mean_exec_time_ns
    STATS["profile_json"] = br.profile_json

    o1 = np.stack([br.results[c]["o1"] for c in range(N_CORES)]).reshape(C_OUT, 128, B, ORI)
    o2 = np.stack([br.results[c]["o2"] for c in range(N_CORES)]).reshape(C_OUT, 64, B, ORI)
    out1 = np.ascontiguousarray(o1[:, :64].transpose(2, 0, 1, 3).astype(np.float32))
    small = np.ascontiguousarray(o1[:, 64:].transpose(2, 0, 1, 3).astype(np.float32))
    out2 = np.ascontiguousarray(o2.transpose(2, 0, 3, 1).astype(np.float32))
    return out1, out2, small

